# revision 71
# baseline (speedup 1.0000x reference)
"""Trainium2 Bass kernel for nn_DistanceLoss (contrastive loss over cosine
similarity matrices).

Math restructure (vs the reference):
  loss = [ sum_i i*ld[i] - sum_{i>j} pos[i,j] ] / n_terms
where ld[i] = log sum_k exp(neg[i,k]).  pos = (p1 @ p1.T)/T is symmetric
with diagonal 1/T, so the strict-lower-triangular sum collapses to
  ( ||sum_i p1_i||^2 / T - B/T ) / 2,
needing only the column-sum s of normalized batch1 -- the [B,B] pos matmul
is eliminated.  Only neg = p1n @ p2n.T needs real compute.

Sharding (4x2): core (g, h) takes batch1 rows [g*1024,(g+1)*1024) and
batch2 rows [h*2048,(h+1)*2048): 6MB of input DMA per core instead of 9MB
for the 1D row split.  Each core emits partial denominators
  part[i] = sum_{k in its half} exp(neg[i,k])
for its 1024 rows plus the partial column-sum s of p1n; the host adds the
two k-halves, takes ln, and does the (tiny) final reduction in float64.

Per-core pipeline:
  - batch1 + first half of batch2 land fp32->bf16 via SWDGE cast-DMA; the
    second half of batch2 goes raw fp32 over the HWDGE (sync) queue in
    parallel and is cast to bf16 on the otherwise-idle Pool engine
  - 4-rows-per-partition interleaved layout = 8KB contiguous reads per
    descriptor; the implied row/k permutation is harmless (exp row-sums
    are permutation invariant; the host unpermutes per-row outputs)
  - batch1 is never normalized on device: raw bf16 slices transpose via
    identity-rhs PE matmuls straight to fp8 p1T, and 1/||b1_i|| folds into
    the per-partition `scale` AP of the main Exp activation
  - batch2: DVE sum-of-squares -> ACT Ln/Exp -> 10/||row|| -> diag-scaled
    PE transpose (normalize+transpose in one matmul) -> fp8 b2sT
  - main matmul fp8 DoubleRow (4x bf16 throughput), PSUM fp32; Exp with
    accum_out = fused row-sums; three k-passes (1024/512/512) so the last
    DMA chunk only owes a short exp tail
  - the s column-sum matmuls are emitted last so the PE does them in the
    shadow of the final exp chain
  - a single explicit LoadActFuncSet of the combined exp+ln+copy table
    (the automatic chooser otherwise thrashes 11 table loads)
"""

import numpy as np
import ml_dtypes

B = 4096
C = 512
NCORES = 8
G = 4                     # batch1 row groups
H = 2                     # batch2 row groups
R1 = B // G               # 1024 batch1 rows per core
R2 = B // H               # 2048 batch2 rows per core
F = 4                     # rows interleaved per partition line
M1 = R1 // 512            # 2 b1 512-row blocks
Q2 = R2 // 512            # 4 b2 512-row chunks
NS1 = R1 // 128           # 8 b1 slices
NS2 = R2 // 128           # 16 b2 slices
CC = C // 128             # 4 contraction chunks
TEMP = 0.1
N_TERMS = B * (B - 1) // 2
NP = 3                    # exp passes: q0+q1 (1024 wide), q2, q3 (512 each)
KRANGES = [(0, 1024), (1024, 1536), (1536, 2048)]

# Fast-exp (Schraudolph float-bit trick) constants for the last pass, which
# runs on the otherwise-idle DVE while ACT finishes pass P1:
#   int32(x*invn1*2^23*log2e + (127*2^23 - C)) bitcast to fp32 ~= exp(x*invn1)
# C = 486411 centers the periodic error so the 512-term row-sum bias is
# ~0.02% (measured), far inside the 2e-2 tolerance.
FE_A = float((1 << 23) * 1.4426950408889634)
FE_B = float(127 * (1 << 23) - 486411)

USE_HWDGE = False         # measured slower both ways: HWDGE shares the 16
                          # DMA engines with SWDGE (no extra bandwidth, it
                          # just delays the serial stream) and the Pool
                          # bf16 cast is ~3x slower than its cost model

_CACHE = {}


def build_bass():
    import concourse.bass as bass
    import concourse.bacc as bacc
    import concourse.tile as tile
    from concourse import mybir
    from contextlib import ExitStack

    fp32 = mybir.dt.float32
    bf16 = mybir.dt.bfloat16
    fp8 = mybir.dt.float8e4
    i32 = mybir.dt.int32
    AF = mybir.ActivationFunctionType
    ALU = mybir.AluOpType
    AX = mybir.AxisListType

    nc = bacc.Bacc("TRN2", target_bir_lowering=False, debug=False,
                   num_devices=NCORES)

    b1s = nc.dram_tensor("b1s", [R1, C], fp32, kind="ExternalInput")
    b2s = nc.dram_tensor("b2s", [R2, C], fp32, kind="ExternalInput")
    ident = nc.dram_tensor("ident", [128, 128], bf16, kind="ExternalInput")
    out = nc.dram_tensor("out", [128, 12], fp32, kind="ExternalOutput")

    with tile.TileContext(nc) as tc, ExitStack() as ctx:
        sb = ctx.enter_context(tc.tile_pool(name="sb", bufs=1))
        dumps = ctx.enter_context(tc.tile_pool(name="dumps", bufs=3))
        pt = ctx.enter_context(tc.tile_pool(name="pt", bufs=3, space="PSUM"))
        pneg = ctx.enter_context(tc.tile_pool(name="pneg", bufs=2, space="PSUM"))
        ps = ctx.enter_context(tc.tile_pool(name="ps", bufs=1, space="PSUM"))

        b1n = sb.tile([128, M1, F, C], bf16, name="b1n")
        b2n = sb.tile([128, Q2, F, C], bf16, name="b2n")
        identb = sb.tile([128, 128], bf16, name="identb")
        p1T = sb.tile([128, CC, R1], fp8, name="p1T")
        b2sT = sb.tile([128, CC, R2], fp8, name="b2sT")
        diag2 = sb.tile([128, NS2, 128], bf16, name="diag2")
        ssq1 = sb.tile([128, NS1], fp32, name="ssq1")
        ssq2 = sb.tile([128, NS2], fp32, name="ssq2")
        ln1 = sb.tile([128, NS1], fp32, name="ln1")
        ln2 = sb.tile([128, NS2], fp32, name="ln2")
        invn1 = sb.tile([128, NS1], fp32, name="invn1")
        invn1b = sb.tile([128, NS1], bf16, name="invn1b")
        invn1s = sb.tile([128, NS1], fp32, name="invn1s")
        invn2 = sb.tile([128, NS2], fp32, name="invn2")
        denoms = sb.tile([128, NS1, NP], fp32, name="denoms")
        outbuf = sb.tile([128, 12], fp32, name="outbuf")
        if USE_HWDGE:
            b2f32 = sb.tile([128, F, C], fp32, name="b2f32")

        # ---- input DMA ---------------------------------------------------
        # SWDGE (gpsimd, casting) carries b1 + b2 q0/q1; the sync HWDGE
        # queue streams b2 q2/q3 as raw fp32 in parallel, cast to bf16 on
        # the Pool engine (which is otherwise idle after issuing descriptors).
        nc.sync.dma_start(identb[:, :], ident.ap())
        b1src = b1s.ap().rearrange("(m p f) c -> p m f c", p=128, f=F)
        b2src = b2s.ap().rearrange("(q p f) c -> p q f c", p=128, f=F)
        # SWDGE stream order = consumption order: b1-m0 (split so the first
        # transposes start sooner), q0, q1 (pass P0 needs both), then b1-m1
        # (only gates the m4-7 half of P0), then q2, q3.
        nc.gpsimd.dma_start(b1n[:, 0, 0:2, :], b1src[:, 0, 0:2, :])
        nc.gpsimd.dma_start(b1n[:, 0, 2:4, :], b1src[:, 0, 2:4, :])
        nc.gpsimd.dma_start(b2n[:, 0, :, :], b2src[:, 0, :, :])
        nc.gpsimd.dma_start(b2n[:, 1, :, :], b2src[:, 1, :, :])
        nc.gpsimd.dma_start(b1n[:, 1, :, :], b1src[:, 1, :, :])
        nc.gpsimd.dma_start(b2n[:, 2, :, :], b2src[:, 2, :, :])
        nc.gpsimd.dma_start(b2n[:, 3, :, :], b2src[:, 3, :, :])

        # ---- batch1: raw transposes (identity rhs) + stats ---------------
        def b1_transpose(s1):
            # s0-3 evacuate on ACT (idle early); s4-7 on DVE, emitted after
            # q2's stats so they fall into DVE's in-order hole while it
            # waits for the q2 transposes -- off ACT's saturated exp spine
            m, f = s1 // F, s1 % F
            ptile = pt.tile([128, CC, 128], fp32, name="pt1", tag="pt")
            for cc in range(CC):
                nc.tensor.matmul(
                    ptile[:, cc, :],
                    lhsT=b1n[:, m, f, cc * 128:(cc + 1) * 128],
                    rhs=identb[:, :], start=True, stop=True)
            ev = nc.scalar.copy if s1 < F else nc.vector.tensor_copy
            ev(p1T[:, :, s1 * 128:(s1 + 1) * 128], ptile[:, :, :])

        def b1_stats(m):
            sl = slice(m * F, (m + 1) * F)
            for f in range(F):
                s1 = m * F + f
                dmp = dumps.tile([128, C], bf16, name="d1", tag="d1")
                nc.vector.scalar_tensor_tensor(
                    out=dmp[:, :], in0=b1n[:, m, f, :], scalar=1.0,
                    in1=b1n[:, m, f, :], op0=ALU.mult, op1=ALU.mult,
                    accum_out=ssq1[:, s1:s1 + 1])
            # invn1 = 1/||row|| = exp(-0.5*ln(ssq))
            nc.scalar.activation(ln1[:, sl], ssq1[:, sl], AF.Ln)
            nc.scalar.activation(invn1[:, sl], ln1[:, sl], AF.Exp, scale=-0.5)
            nc.vector.tensor_scalar(invn1b[:, sl], invn1[:, sl], 1.0, None,
                                    op0=ALU.mult)
            nc.vector.tensor_scalar(invn1s[:, sl], invn1[:, sl], FE_A, None,
                                    op0=ALU.mult)

        # ---- batch2 per-chunk stats + diag + transpose -------------------
        def b2_stats(q):
            for f in range(F):
                s2 = q * F + f
                dmp = dumps.tile([128, C], bf16, name="d2", tag="d2")
                nc.vector.scalar_tensor_tensor(
                    out=dmp[:, :], in0=b2n[:, q, f, :], scalar=1.0,
                    in1=b2n[:, q, f, :], op0=ALU.mult, op1=ALU.mult,
                    accum_out=ssq2[:, s2:s2 + 1])
            sl = slice(q * F, (q + 1) * F)
            # 10/||row|| = exp(-0.5*ln(0.01*ssq))
            nc.scalar.activation(ln2[:, sl], ssq2[:, sl], AF.Ln, scale=0.01)
            nc.scalar.activation(invn2[:, sl], ln2[:, sl], AF.Exp, scale=-0.5)
            for f in range(F):
                s2 = q * F + f
                # diag build on the otherwise-idle Pool engine: identb *
                # invn2[p] as tensor_tensor with a stride-0 broadcast AP
                # (Pool rejects scalar-AP tensor_scalar, but takes this)
                nc.gpsimd.tensor_tensor(
                    out=diag2[:, s2, :], in0=identb[:, :],
                    in1=invn2[:, s2:s2 + 1].broadcast_to([128, 128]),
                    op=ALU.mult)

        def b2_transpose(q):
            # q0/q1 evacuations on ACT (idle early); q2/q3 on DVE, whose
            # mid-kernel window is free once the transposes are hoisted
            # ahead of the pneg-paced P0 m4-7 mains
            ev = nc.scalar.copy if q < 2 else nc.vector.tensor_copy
            for f in range(F):
                s2 = q * F + f
                ptile = pt.tile([128, CC, 128], fp32, name="pt2", tag="pt")
                for cc in range(CC):
                    nc.tensor.matmul(
                        ptile[:, cc, :],
                        lhsT=b2n[:, q, f, cc * 128:(cc + 1) * 128],
                        rhs=diag2[:, s2, :], start=True, stop=True)
                ev(b2sT[:, :, s2 * 128:(s2 + 1) * 128], ptile[:, :, :])

        # ---- main matmul + fused exp/rowsum ------------------------------
        def main_pass(P, ms, fast_ms=()):
            k0, k1 = KRANGES[P]
            ngrp = (k1 - k0) // 512
            for m in ms:
                fast = m in fast_ms
                ntile = pneg.tile([128, 2, 512], fp32, name="ntile", tag="pn")
                for kg in range(2):
                    for mgx in range(ngrp):
                        nc.tensor.matmul(
                            ntile[:, mgx, :],
                            lhsT=p1T[:, 2 * kg:2 * kg + 2, m * 128:(m + 1) * 128],
                            rhs=b2sT[:, 2 * kg:2 * kg + 2,
                                     k0 + mgx * 512:k0 + (mgx + 1) * 512],
                            start=(kg == 0), stop=(kg == 1),
                            perf_mode=mybir.MatmulPerfMode.DoubleRow)
                if fast:
                    # exp+rowsum on DVE (bit-trick), in ACT's shadow
                    ti = dumps.tile([128, 512], i32, name="fe", tag="fe")
                    nc.vector.tensor_scalar(
                        ti[:, :], ntile[:, 0, :], invn1s[:, m:m + 1], FE_B,
                        op0=ALU.mult, op1=ALU.add)
                    nc.vector.tensor_reduce(
                        denoms[:, m, P:P + 1], ti[:, :].bitcast(fp32),
                        axis=AX.X, op=ALU.add)
                    continue
                dmp = dumps.tile([128, 1024], bf16, name="de", tag="de")
                src = ntile[:, :, :].rearrange("p a b -> p (a b)") if ngrp == 2 \
                    else ntile[:, 0, :]
                nc.scalar.activation(
                    dmp[:, 0:ngrp * 512], src,
                    AF.Exp, scale=invn1[:, m:m + 1],
                    accum_out=denoms[:, m, P:P + 1])

        def s_matmuls(psum_s, s1s):
            # psum_s[c] = sum_i p1n[i, c], accumulated over two emission
            # sites placed in the PE's measured slack holes (waiting for q1
            # DMA, and the pneg-paced gap before the P1 mains)
            for s1 in s1s:
                m, f = s1 // F, s1 % F
                for cc in range(CC):
                    nc.tensor.matmul(
                        psum_s[:, cc:cc + 1],
                        lhsT=b1n[:, m, f, cc * 128:(cc + 1) * 128],
                        rhs=invn1b[:, s1:s1 + 1],
                        start=(s1 == 0), stop=(s1 == NS1 - 1))

        # ---- emission order (per-engine program order = pipeline) --------
        # chunk processing follows DMA arrival: b1m0, q0, q1, b1m1, q2, q3
        psum_s = ps.tile([128, CC], fp32, name="psum_s", tag="ps")
        for s1 in range(0, F):
            b1_transpose(s1)
        b1_stats(0)
        s_matmuls(psum_s, range(0, F))   # fills PE's hole waiting for q0/q1
        b2_stats(0)
        b2_transpose(0)
        b2_stats(1)
        b2_transpose(1)
        main_pass(0, range(0, F))
        # q2's stats go first in DVE order (data lands before b1-m1's
        # consumers need anything), then m1 stats, then the b1 s4-7
        # transposes whose DVE evacs fill the wait-for-q2T hole
        b2_stats(2)
        b1_stats(1)
        for s1 in range(F, NS1):
            b1_transpose(s1)
        # m1-half of the s accumulation: the P0 m4-7 mains it delays are
        # ACT-drain-paced anyway, and this keeps s clear of the q2 path
        # and the endgame
        s_matmuls(psum_s, range(F, NS1))
        b2_transpose(2)
        main_pass(0, range(F, NS1))
        b2_stats(3)
        b2_transpose(3)
        # last two passes drain on BOTH engines: DVE fast-exps middle/late
        # tiles while ACT keeps the LAST tiles of the final pass -- its
        # per-tile drain (646ns) beats DVE's TS+reduce (1156ns), so the op
        # chained after the last main is the fast consumer
        main_pass(1, range(0, NS1), fast_ms={4, 5, 6, 7})
        main_pass(2, range(0, NS1), fast_ms={3, 4, 5})

        # ---- epilogue ----------------------------------------------------
        nc.vector.tensor_copy(outbuf[:, 8:12], psum_s[:, :])
        nc.vector.tensor_reduce(outbuf[:, 0:8], denoms[:, :, :],
                                axis=AX.X, op=ALU.add)
        nc.sync.dma_start(out.ap(), outbuf[:, :])

    # Pin the combined exp+ln+copy activation table before compiling: the
    # automatic chooser alternates natural_log/exp_and_others and inserts a
    # 1.3us table load around every Ln<->Exp transition otherwise.
    try:
        from concourse.hw_specs import get_activation_tables
        tables = get_activation_tables(nc.m.arch)
        set_id = next(
            i for i, (_, fns) in enumerate(tables.items())
            if {AF.Exp, AF.Ln, AF.Copy} <= fns)
    except Exception:
        set_id = 6  # natural_log_exp_and_others in the shipped act_info.json
    inst = mybir.InstLoadActFuncSet(
        name=nc.get_next_instruction_name(), ins=[], outs=[],
        act_func_set_id=set_id)
    inst.engine = mybir.EngineType.Activation
    nc.register_instruction(inst)
    nc.main_func.blocks[0].instructions.insert(0, inst)

    nc.compile()
    return nc


def _get_nc():
    if "nc" not in _CACHE:
        _CACHE["nc"] = build_bass()
    return _CACHE["nc"]


def make_in_maps(batch1, batch2):
    batch1 = np.ascontiguousarray(np.asarray(batch1, dtype=np.float32))
    batch2 = np.ascontiguousarray(np.asarray(batch2, dtype=np.float32))
    eye = np.eye(128, dtype=ml_dtypes.bfloat16)
    maps = []
    for c in range(NCORES):
        g, h = c // H, c % H
        maps.append({
            "b1s": np.ascontiguousarray(batch1[g * R1:(g + 1) * R1]),
            "b2s": np.ascontiguousarray(batch2[h * R2:(h + 1) * R2]),
            "ident": eye,
        })
    return maps


def _row_perm():
    # out[p, s1] corresponds to local row (s1//F)*512 + 4*p + (s1%F)
    p = np.arange(128)
    s1 = np.arange(NS1)
    return (s1[None, :] // F) * 512 + 4 * p[:, None] + (s1[None, :] % F)


def combine(results):
    rows = _row_perm()
    D = np.zeros((H, B), dtype=np.float64)
    s = np.zeros(C, dtype=np.float64)
    for c in range(NCORES):
        g, h = c // H, c % H
        o = np.asarray(results[c]["out"], np.float64)  # [128, 12]
        idx = g * R1 + rows
        D[h, idx.ravel()] += o[:, 0:NS1].ravel()
        if h == 0:
            # s[cc*128 + p] = o[p, 8+cc]
            s += o[:, 8:12].T.ravel()
    ld = np.log(D[0] + D[1])
    term1 = np.dot(np.arange(B, dtype=np.float64), ld)
    tri = (np.dot(s, s) / TEMP - B / TEMP) / 2.0
    return np.asarray((term1 - tri) / N_TERMS, dtype=np.float32)


def run_hw(in_maps, trace=False, **kwargs):
    from concourse.bass_utils import run_bass_kernel_spmd
    return run_bass_kernel_spmd(_get_nc(), in_maps,
                                core_ids=list(range(NCORES)),
                                trace=trace, **kwargs)


def kernel(batch1, batch2):
    res = run_hw(make_in_maps(batch1, batch2))
    return combine(res.results)


# revision 72
# speedup vs baseline: 1.2766x; 1.2766x over previous
"""Trainium2 Bass kernel for nn_DistanceLoss (contrastive loss over cosine
similarity matrices).

Math restructure (vs the reference):
  loss = [ sum_i i*ld[i] - sum_{i>j} pos[i,j] ] / n_terms
where ld[i] = log sum_k exp(neg[i,k]).  pos = (p1 @ p1.T)/T is symmetric
with diagonal 1/T, so the strict-lower-triangular sum collapses to
  ( ||sum_i p1_i||^2 / T - B/T ) / 2,
needing only the column-sum s of normalized batch1 -- the [B,B] pos matmul
is eliminated.  Only neg = p1n @ p2n.T needs real compute.

Sharding (4x2): core (g, h) takes batch1 rows [g*1024,(g+1)*1024) and
batch2 rows [h*2048,(h+1)*2048): 6MB of input DMA per core instead of 9MB
for the 1D row split.  Each core emits partial denominators
  part[i] = sum_{k in its half} exp(neg[i,k])
for its 1024 rows plus the partial column-sum s of p1n; the host adds the
two k-halves, takes ln, and does the (tiny) final reduction in float64.

Per-core pipeline:
  - batch1 + first half of batch2 land fp32->bf16 via SWDGE cast-DMA; the
    second half of batch2 goes raw fp32 over the HWDGE (sync) queue in
    parallel and is cast to bf16 on the otherwise-idle Pool engine
  - 4-rows-per-partition interleaved layout = 8KB contiguous reads per
    descriptor; the implied row/k permutation is harmless (exp row-sums
    are permutation invariant; the host unpermutes per-row outputs)
  - batch1 is never normalized on device: raw bf16 slices transpose via
    identity-rhs PE matmuls straight to fp8 p1T, and 1/||b1_i|| folds into
    the per-partition `scale` AP of the main Exp activation
  - batch2: DVE sum-of-squares -> ACT Ln/Exp -> 10/||row|| -> diag-scaled
    PE transpose (normalize+transpose in one matmul) -> fp8 b2sT
  - main matmul fp8 DoubleRow (4x bf16 throughput), PSUM fp32; Exp with
    accum_out = fused row-sums; three k-passes (1024/512/512) so the last
    DMA chunk only owes a short exp tail
  - the s column-sum matmuls are emitted last so the PE does them in the
    shadow of the final exp chain
  - a single explicit LoadActFuncSet of the combined exp+ln+copy table
    (the automatic chooser otherwise thrashes 11 table loads)
"""

import numpy as np
import ml_dtypes

B = 4096
C = 512
NCORES = 8
G = 4                     # batch1 row groups
H = 2                     # batch2 row groups
R1 = B // G               # 1024 batch1 rows per core
R2 = B // H               # 2048 batch2 rows per core
F = 4                     # rows interleaved per partition line
M1 = R1 // 512            # 2 b1 512-row blocks
Q2 = R2 // 512            # 4 b2 512-row chunks
NS1 = R1 // 128           # 8 b1 slices
NS2 = R2 // 128           # 16 b2 slices
CC = C // 128             # 4 contraction chunks
TEMP = 0.1
N_TERMS = B * (B - 1) // 2
NP = 3                    # exp passes: q0+q1 (1024 wide), q2, q3 (512 each)
KRANGES = [(0, 1024), (1024, 1536), (1536, 2048)]

# Fast-exp (Schraudolph float-bit trick) constants for the last pass, which
# runs on the otherwise-idle DVE while ACT finishes pass P1:
#   int32(x*invn1*2^23*log2e + (127*2^23 - C)) bitcast to fp32 ~= exp(x*invn1)
# C = 486411 centers the periodic error so the 512-term row-sum bias is
# ~0.02% (measured), far inside the 2e-2 tolerance.
FE_A = float((1 << 23) * 1.4426950408889634)
FE_B = float(127 * (1 << 23) - 486411)

USE_HWDGE = False         # measured slower both ways: HWDGE shares the 16
                          # DMA engines with SWDGE (no extra bandwidth, it
                          # just delays the serial stream) and the Pool
                          # bf16 cast is ~3x slower than its cost model

_CACHE = {}


def build_bass():
    import concourse.bass as bass
    import concourse.bacc as bacc
    import concourse.tile as tile
    from concourse import mybir
    from contextlib import ExitStack

    fp32 = mybir.dt.float32
    bf16 = mybir.dt.bfloat16
    fp8 = mybir.dt.float8e4
    i32 = mybir.dt.int32
    AF = mybir.ActivationFunctionType
    ALU = mybir.AluOpType
    AX = mybir.AxisListType

    nc = bacc.Bacc("TRN2", target_bir_lowering=False, debug=False,
                   num_devices=NCORES)

    b1s = nc.dram_tensor("b1s", [R1, C], fp32, kind="ExternalInput")
    b2s = nc.dram_tensor("b2s", [R2, C], fp32, kind="ExternalInput")
    ident = nc.dram_tensor("ident", [128, 128], bf16, kind="ExternalInput")
    out = nc.dram_tensor("out", [128, 12], fp32, kind="ExternalOutput")

    with tile.TileContext(nc) as tc, ExitStack() as ctx:
        sb = ctx.enter_context(tc.tile_pool(name="sb", bufs=1))
        dumps = ctx.enter_context(tc.tile_pool(name="dumps", bufs=3))
        pt = ctx.enter_context(tc.tile_pool(name="pt", bufs=3, space="PSUM"))
        pneg = ctx.enter_context(tc.tile_pool(name="pneg", bufs=2, space="PSUM"))
        ps = ctx.enter_context(tc.tile_pool(name="ps", bufs=1, space="PSUM"))

        b1n = sb.tile([128, M1, F, C], bf16, name="b1n")
        b2n = sb.tile([128, Q2, F, C], bf16, name="b2n")
        identb = sb.tile([128, 128], bf16, name="identb")
        p1T = sb.tile([128, CC, R1], fp8, name="p1T")
        b2sT = sb.tile([128, CC, R2], fp8, name="b2sT")
        diag2 = sb.tile([128, NS2, 128], bf16, name="diag2")
        ssq1 = sb.tile([128, NS1], fp32, name="ssq1")
        ssq2 = sb.tile([128, NS2], fp32, name="ssq2")
        ln1 = sb.tile([128, NS1], fp32, name="ln1")
        ln2 = sb.tile([128, NS2], fp32, name="ln2")
        invn1 = sb.tile([128, NS1], fp32, name="invn1")
        invn1b = sb.tile([128, NS1], bf16, name="invn1b")
        invn1s = sb.tile([128, NS1], fp32, name="invn1s")
        invn2 = sb.tile([128, NS2], fp32, name="invn2")
        denoms = sb.tile([128, NS1, NP], fp32, name="denoms")
        outbuf = sb.tile([128, 12], fp32, name="outbuf")
        if USE_HWDGE:
            b2f32 = sb.tile([128, F, C], fp32, name="b2f32")

        # ---- input DMA ---------------------------------------------------
        # SWDGE (gpsimd, casting) carries b1 + b2 q0/q1; the sync HWDGE
        # queue streams b2 q2/q3 as raw fp32 in parallel, cast to bf16 on
        # the Pool engine (which is otherwise idle after issuing descriptors).
        nc.sync.dma_start(identb[:, :], ident.ap())
        b1src = b1s.ap().rearrange("(m p f) c -> p m f c", p=128, f=F)
        b2src = b2s.ap().rearrange("(q p f) c -> p q f c", p=128, f=F)
        # SWDGE stream order = consumption order: b1-m0 (split so the first
        # transposes start sooner), q0, q1 (pass P0 needs both), then b1-m1
        # (only gates the m4-7 half of P0), then q2, q3.
        nc.gpsimd.dma_start(b1n[:, 0, 0:2, :], b1src[:, 0, 0:2, :])
        nc.gpsimd.dma_start(b1n[:, 0, 2:4, :], b1src[:, 0, 2:4, :])
        nc.gpsimd.dma_start(b2n[:, 0, :, :], b2src[:, 0, :, :])
        nc.gpsimd.dma_start(b2n[:, 1, :, :], b2src[:, 1, :, :])
        nc.gpsimd.dma_start(b1n[:, 1, :, :], b1src[:, 1, :, :])
        nc.gpsimd.dma_start(b2n[:, 2, :, :], b2src[:, 2, :, :])
        nc.gpsimd.dma_start(b2n[:, 3, :, :], b2src[:, 3, :, :])

        # ---- batch1: raw transposes (identity rhs) + stats ---------------
        def b1_transpose(s1):
            # s0-3 evacuate on ACT (idle early); s4-7 on DVE, emitted after
            # q2's stats so they fall into DVE's in-order hole while it
            # waits for the q2 transposes -- off ACT's saturated exp spine
            m, f = s1 // F, s1 % F
            ptile = pt.tile([128, CC, 128], fp32, name="pt1", tag="pt")
            for cc in range(CC):
                nc.tensor.matmul(
                    ptile[:, cc, :],
                    lhsT=b1n[:, m, f, cc * 128:(cc + 1) * 128],
                    rhs=identb[:, :], start=True, stop=True)
            ev = nc.scalar.copy if s1 < F else nc.vector.tensor_copy
            ev(p1T[:, :, s1 * 128:(s1 + 1) * 128], ptile[:, :, :])

        def b1_stats(m):
            sl = slice(m * F, (m + 1) * F)
            for f in range(F):
                s1 = m * F + f
                dmp = dumps.tile([128, C], bf16, name="d1", tag="d1")
                nc.vector.scalar_tensor_tensor(
                    out=dmp[:, :], in0=b1n[:, m, f, :], scalar=1.0,
                    in1=b1n[:, m, f, :], op0=ALU.mult, op1=ALU.mult,
                    accum_out=ssq1[:, s1:s1 + 1])
            # invn1 = 1/||row|| = exp(-0.5*ln(ssq))
            nc.scalar.activation(ln1[:, sl], ssq1[:, sl], AF.Ln)
            nc.scalar.activation(invn1[:, sl], ln1[:, sl], AF.Exp, scale=-0.5)
            nc.vector.tensor_scalar(invn1b[:, sl], invn1[:, sl], 1.0, None,
                                    op0=ALU.mult)
            nc.vector.tensor_scalar(invn1s[:, sl], invn1[:, sl], FE_A, None,
                                    op0=ALU.mult)

        # ---- batch2 per-chunk stats + diag + transpose -------------------
        def b2_stats(q):
            for f in range(F):
                s2 = q * F + f
                dmp = dumps.tile([128, C], bf16, name="d2", tag="d2")
                nc.vector.scalar_tensor_tensor(
                    out=dmp[:, :], in0=b2n[:, q, f, :], scalar=1.0,
                    in1=b2n[:, q, f, :], op0=ALU.mult, op1=ALU.mult,
                    accum_out=ssq2[:, s2:s2 + 1])
            sl = slice(q * F, (q + 1) * F)
            # 10/||row|| = exp(-0.5*ln(0.01*ssq))
            nc.scalar.activation(ln2[:, sl], ssq2[:, sl], AF.Ln, scale=0.01)
            nc.scalar.activation(invn2[:, sl], ln2[:, sl], AF.Exp, scale=-0.5)
            for f in range(F):
                s2 = q * F + f
                nc.vector.tensor_scalar_mul(
                    diag2[:, s2, :], identb[:, :], invn2[:, s2:s2 + 1])

        def b2_transpose(q):
            # q0/q1 evacuations on ACT (idle early); q2/q3 on DVE, whose
            # mid-kernel window is free once the transposes are hoisted
            # ahead of the pneg-paced P0 m4-7 mains
            ev = nc.scalar.copy if q < 2 else nc.vector.tensor_copy
            for f in range(F):
                s2 = q * F + f
                ptile = pt.tile([128, CC, 128], fp32, name="pt2", tag="pt")
                for cc in range(CC):
                    nc.tensor.matmul(
                        ptile[:, cc, :],
                        lhsT=b2n[:, q, f, cc * 128:(cc + 1) * 128],
                        rhs=diag2[:, s2, :], start=True, stop=True)
                ev(b2sT[:, :, s2 * 128:(s2 + 1) * 128], ptile[:, :, :])

        # ---- main matmul + fused exp/rowsum ------------------------------
        def main_pass(P, ms, fast_ms=()):
            k0, k1 = KRANGES[P]
            ngrp = (k1 - k0) // 512
            for m in ms:
                fast = m in fast_ms
                ntile = pneg.tile([128, 2, 512], fp32, name="ntile", tag="pn")
                for kg in range(2):
                    for mgx in range(ngrp):
                        nc.tensor.matmul(
                            ntile[:, mgx, :],
                            lhsT=p1T[:, 2 * kg:2 * kg + 2, m * 128:(m + 1) * 128],
                            rhs=b2sT[:, 2 * kg:2 * kg + 2,
                                     k0 + mgx * 512:k0 + (mgx + 1) * 512],
                            start=(kg == 0), stop=(kg == 1),
                            perf_mode=mybir.MatmulPerfMode.DoubleRow)
                if fast:
                    # exp+rowsum on DVE (bit-trick), in ACT's shadow
                    ti = dumps.tile([128, 512], i32, name="fe", tag="fe")
                    nc.vector.tensor_scalar(
                        ti[:, :], ntile[:, 0, :], invn1s[:, m:m + 1], FE_B,
                        op0=ALU.mult, op1=ALU.add)
                    nc.vector.tensor_reduce(
                        denoms[:, m, P:P + 1], ti[:, :].bitcast(fp32),
                        axis=AX.X, op=ALU.add)
                    continue
                dmp = dumps.tile([128, 1024], bf16, name="de", tag="de")
                src = ntile[:, :, :].rearrange("p a b -> p (a b)") if ngrp == 2 \
                    else ntile[:, 0, :]
                nc.scalar.activation(
                    dmp[:, 0:ngrp * 512], src,
                    AF.Exp, scale=invn1[:, m:m + 1],
                    accum_out=denoms[:, m, P:P + 1])

        def s_matmuls(psum_s, s1s):
            # psum_s[c] = sum_i p1n[i, c], accumulated over two emission
            # sites placed in the PE's measured slack holes (waiting for q1
            # DMA, and the pneg-paced gap before the P1 mains)
            for s1 in s1s:
                m, f = s1 // F, s1 % F
                for cc in range(CC):
                    nc.tensor.matmul(
                        psum_s[:, cc:cc + 1],
                        lhsT=b1n[:, m, f, cc * 128:(cc + 1) * 128],
                        rhs=invn1b[:, s1:s1 + 1],
                        start=(s1 == 0), stop=(s1 == NS1 - 1))

        # ---- emission order (per-engine program order = pipeline) --------
        # chunk processing follows DMA arrival: b1m0, q0, q1, b1m1, q2, q3
        psum_s = ps.tile([128, CC], fp32, name="psum_s", tag="ps")
        for s1 in range(0, F):
            b1_transpose(s1)
        b1_stats(0)
        s_matmuls(psum_s, range(0, F))   # fills PE's hole waiting for q0/q1
        b2_stats(0)
        b2_transpose(0)
        b2_stats(1)
        b2_transpose(1)
        main_pass(0, range(0, F))
        # q2's stats go first in DVE order (data lands before b1-m1's
        # consumers need anything), then m1 stats, then the b1 s4-7
        # transposes whose DVE evacs fill the wait-for-q2T hole
        b2_stats(2)
        b1_stats(1)
        for s1 in range(F, NS1):
            b1_transpose(s1)
        # m1-half of the s accumulation: the P0 m4-7 mains it delays are
        # ACT-drain-paced anyway, and this keeps s clear of the q2 path
        # and the endgame
        s_matmuls(psum_s, range(F, NS1))
        b2_transpose(2)
        main_pass(0, range(F, NS1))
        b2_stats(3)
        b2_transpose(3)
        # last two passes drain on BOTH engines: DVE fast-exps middle/late
        # tiles while ACT keeps the LAST tiles of the final pass -- its
        # per-tile drain (646ns) beats DVE's TS+reduce (1156ns), so the op
        # chained after the last main is the fast consumer
        main_pass(1, range(0, NS1), fast_ms={4, 5, 6, 7})
        main_pass(2, range(0, NS1), fast_ms={3, 4, 5})

        # ---- epilogue ----------------------------------------------------
        nc.vector.tensor_copy(outbuf[:, 8:12], psum_s[:, :])
        nc.vector.tensor_reduce(outbuf[:, 0:8], denoms[:, :, :],
                                axis=AX.X, op=ALU.add)
        nc.sync.dma_start(out.ap(), outbuf[:, :])

    # Pin the combined exp+ln+copy activation table before compiling: the
    # automatic chooser alternates natural_log/exp_and_others and inserts a
    # 1.3us table load around every Ln<->Exp transition otherwise.
    try:
        from concourse.hw_specs import get_activation_tables
        tables = get_activation_tables(nc.m.arch)
        set_id = next(
            i for i, (_, fns) in enumerate(tables.items())
            if {AF.Exp, AF.Ln, AF.Copy} <= fns)
    except Exception:
        set_id = 6  # natural_log_exp_and_others in the shipped act_info.json
    inst = mybir.InstLoadActFuncSet(
        name=nc.get_next_instruction_name(), ins=[], outs=[],
        act_func_set_id=set_id)
    inst.engine = mybir.EngineType.Activation
    nc.register_instruction(inst)
    nc.main_func.blocks[0].instructions.insert(0, inst)

    nc.compile()
    return nc


def _get_nc():
    if "nc" not in _CACHE:
        _CACHE["nc"] = build_bass()
    return _CACHE["nc"]


def make_in_maps(batch1, batch2):
    batch1 = np.ascontiguousarray(np.asarray(batch1, dtype=np.float32))
    batch2 = np.ascontiguousarray(np.asarray(batch2, dtype=np.float32))
    eye = np.eye(128, dtype=ml_dtypes.bfloat16)
    maps = []
    for c in range(NCORES):
        g, h = c // H, c % H
        maps.append({
            "b1s": np.ascontiguousarray(batch1[g * R1:(g + 1) * R1]),
            "b2s": np.ascontiguousarray(batch2[h * R2:(h + 1) * R2]),
            "ident": eye,
        })
    return maps


def _row_perm():
    # out[p, s1] corresponds to local row (s1//F)*512 + 4*p + (s1%F)
    p = np.arange(128)
    s1 = np.arange(NS1)
    return (s1[None, :] // F) * 512 + 4 * p[:, None] + (s1[None, :] % F)


def combine(results):
    rows = _row_perm()
    D = np.zeros((H, B), dtype=np.float64)
    s = np.zeros(C, dtype=np.float64)
    for c in range(NCORES):
        g, h = c // H, c % H
        o = np.asarray(results[c]["out"], np.float64)  # [128, 12]
        idx = g * R1 + rows
        D[h, idx.ravel()] += o[:, 0:NS1].ravel()
        if h == 0:
            # s[cc*128 + p] = o[p, 8+cc]
            s += o[:, 8:12].T.ravel()
    ld = np.log(D[0] + D[1])
    term1 = np.dot(np.arange(B, dtype=np.float64), ld)
    tri = (np.dot(s, s) / TEMP - B / TEMP) / 2.0
    return np.asarray((term1 - tri) / N_TERMS, dtype=np.float32)


def run_hw(in_maps, trace=False, **kwargs):
    from concourse.bass_utils import run_bass_kernel_spmd
    return run_bass_kernel_spmd(_get_nc(), in_maps,
                                core_ids=list(range(NCORES)),
                                trace=trace, **kwargs)


def kernel(batch1, batch2):
    res = run_hw(make_in_maps(batch1, batch2))
    return combine(res.results)
